# revision 6
# baseline (speedup 1.0000x reference)
"""Trainium2 Bass kernel for CrossAttentionModule (channel-wise attention).

Math restructuring
------------------
Reference (per sample b, with n = H*W pixels, C channels):
    q = Wq @ fm + bq            # [C, n]
    k = Wk * am + bk            # [C, n]  (rank-2 in the channel axis!)
    v = Wv @ fm + bv            # [C, n]
    scores[i, j] = <q[i, :], k[j, :]>
    out = softmax_j(scores) @ v
    result = gamma * out + fm

Because k[j, p] = Wk[j] * am[p] + bk[j]:
    scores[i, j] = s1[i] * Wk[j] + s2[i] * bk[j]
where
    s1 = Wq @ (fm @ am) + sum(am) * bq      # [C]
    s2 = Wq @ (fm @ 1)  + n * bq            # [C]
so the whole Q GEMM and the scores GEMM collapse into two C-vector
matvecs against Wq.  The softmax row max is max_j of a 2D linear
function over the point set {(Wk[j], bk[j])} -- we evaluate it over a
small set of direction-sampled support points (argmax over 64 angles,
precomputed on host from the weights; undershoot <= r*(1-cos(pi/64)),
harmless inside exp).  Z comes for free from the main matmul by
appending a ones-column to v.

Sharding: data-parallel over batch; core b computes sample b.
"""

import os
import sys

for _p in ("/opt/trn_rl_repo", "/root/.axon_site/_ro/trn_rl_repo"):
    if os.path.isdir(_p) and _p not in sys.path:
        sys.path.insert(0, _p)

from contextlib import ExitStack

import numpy as np

import concourse.bacc as bacc
import concourse.bass as bass
import concourse.mybir as mybir
import concourse.tile as tile

C = 2048
NPIX = 1024
NCORES = 8
NH = 64  # direction-sampled support points for the row max
NCHUNK = C // 128  # 16

F32 = mybir.dt.float32
OP = mybir.AluOpType
AX = mybir.AxisListType
AF = mybir.ActivationFunctionType

# dtype used for the two big GEMMs (V and probs@V).  float32r streams at
# full PE rate (4x faster than float32) at reduced multiply precision;
# the score/softmax path always stays full fp32.
MM_DT = mybir.dt.float32r if os.environ.get("CA_MM_DT", "f32r") == "f32r" else F32

# n-chunk split of the 1025-wide (v | ones) moving operand: each matmul
# output must fit one PSUM bank (<=512 fp32).
NSPLIT = [(0, 342), (342, 684), (684, 1025)]


def build_nc(mm_dt=MM_DT):
    nc = bacc.Bacc("TRN2", target_bir_lowering=False)

    fm = nc.declare_dram_parameter("fm", [C, NPIX], F32, isOutput=False)
    am = nc.declare_dram_parameter("am", [1, NPIX], F32, isOutput=False)
    wvt = nc.declare_dram_parameter("wvt", [NCHUNK, C, 128], F32, isOutput=False)
    wqt = nc.declare_dram_parameter("wqt", [NCHUNK, C, 128], F32, isOutput=False)
    wk = nc.declare_dram_parameter("wk", [C, 1], F32, isOutput=False)
    bk = nc.declare_dram_parameter("bk", [C, 1], F32, isOutput=False)
    bq = nc.declare_dram_parameter("bq", [C, 1], F32, isOutput=False)
    bv = nc.declare_dram_parameter("bv", [C, 1], F32, isOutput=False)
    hull = nc.declare_dram_parameter("hull", [2, NH], F32, isOutput=False)
    gam = nc.declare_dram_parameter("gamma", [1, 1], F32, isOutput=False)
    out = nc.declare_dram_parameter("out", [C, NPIX], F32, isOutput=True)

    def mm(ap):
        return ap.bitcast(mm_dt) if mm_dt != F32 else ap

    with ExitStack() as ctx:
        tc = ctx.enter_context(tile.TileContext(nc))
        small = ctx.enter_context(tc.tile_pool(name="small", bufs=1))
        vpool = ctx.enter_context(tc.tile_pool(name="v", bufs=NCHUNK))
        dramp = ctx.enter_context(tc.tile_pool(name="dram", bufs=1, space="DRAM"))

        # ---- small persistent tiles -------------------------------------
        am_bc = small.tile([128, NPIX], F32, tag="am_bc")
        nc.gpsimd.dma_start(out=am_bc[:], in_=am[:].to_broadcast([128, NPIX]))
        hull_wk = small.tile([128, NH], F32, tag="hwk")
        nc.gpsimd.dma_start(out=hull_wk[:], in_=hull[0:1, :].to_broadcast([128, NH]))
        hull_bk = small.tile([128, NH], F32, tag="hbk")
        nc.gpsimd.dma_start(out=hull_bk[:], in_=hull[1:2, :].to_broadcast([128, NH]))
        gam_bc = small.tile([128, 1], F32, tag="gam")
        nc.gpsimd.dma_start(out=gam_bc[:], in_=gam[:].to_broadcast([128, 1]))

        wk_t = small.tile([128, NCHUNK], F32, tag="wk_t")
        bk_t = small.tile([128, NCHUNK], F32, tag="bk_t")
        bq_t = small.tile([128, NCHUNK], F32, tag="bq_t")
        bv_t = small.tile([128, NCHUNK], F32, tag="bv_t")
        for o in range(NCHUNK):
            sl = slice(o * 128, (o + 1) * 128)
            nc.sync.dma_start(out=wk_t[:, o : o + 1], in_=wk[sl, :])
            nc.sync.dma_start(out=bk_t[:, o : o + 1], in_=bk[sl, :])
            nc.sync.dma_start(out=bq_t[:, o : o + 1], in_=bq[sl, :])
            nc.sync.dma_start(out=bv_t[:, o : o + 1], in_=bv[sl, :])

        a_col = small.tile([128, 1], F32, tag="a_col")
        nc.vector.tensor_reduce(out=a_col[:], in_=am_bc[:], axis=AX.X, op=OP.add)

        # s1 in cols 0..15, s2 in 16..31, m in 32..47 (col o <-> i-chunk o)
        s_cols = small.tile([128, 3 * NCHUNK], F32, tag="s_cols")
        scratch = dramp.tile([3, C], F32, tag="scratch")

        v_tiles = []

        # ---- phase A/B: u reduction, V GEMM, s matvec, row-max ----------
        with ExitStack() as pab:
            fm_pool = pab.enter_context(tc.tile_pool(name="fm", bufs=NCHUNK))
            u_pool = pab.enter_context(tc.tile_pool(name="u", bufs=NCHUNK))
            wv_pool = pab.enter_context(tc.tile_pool(name="wv", bufs=24))
            wq_pool = pab.enter_context(tc.tile_pool(name="wq", bufs=24))
            scr_pool = pab.enter_context(tc.tile_pool(name="scr", bufs=1))
            hx_pool = pab.enter_context(tc.tile_pool(name="hx", bufs=2))
            psv = pab.enter_context(tc.tile_pool(name="psv", bufs=4, space="PSUM"))
            pss = pab.enter_context(tc.tile_pool(name="pss", bufs=2, space="PSUM"))

            fm_tiles = []
            u_tiles = []
            for c in range(NCHUNK):
                ft = fm_pool.tile([128, NPIX], F32, tag="fm")
                nc.sync.dma_start(out=ft[:], in_=fm[c * 128 : (c + 1) * 128, :])
                fm_tiles.append(ft)
                ut = u_pool.tile([128, 2], F32, tag="u")
                scr_a = scr_pool.tile([128, NPIX], F32, tag="scr_a")
                nc.vector.tensor_mul(scr_a[:], ft[:], am_bc[:])
                nc.vector.tensor_reduce(
                    out=ut[:, 0:1], in_=scr_a[:], axis=AX.X, op=OP.add
                )
                nc.vector.tensor_reduce(
                    out=ut[:, 1:2], in_=ft[:], axis=AX.X, op=OP.add
                )
                u_tiles.append(ut)

            for o in range(NCHUNK):
                wv_tiles = []
                for c in range(NCHUNK):
                    wt = wv_pool.tile([128, 128], F32, tag="wv")
                    nc.sync.dma_start(
                        out=wt[:], in_=wvt[o, c * 128 : (c + 1) * 128, :]
                    )
                    wv_tiles.append(wt)
                pv0 = psv.tile([128, 512], F32, tag="pv")
                pv1 = psv.tile([128, 512], F32, tag="pv")
                for c in range(NCHUNK):
                    nc.tensor.matmul(
                        pv0[:],
                        mm(wv_tiles[c][:]),
                        mm(fm_tiles[c][:, 0:512]),
                        start=(c == 0),
                        stop=(c == NCHUNK - 1),
                    )
                for c in range(NCHUNK):
                    nc.tensor.matmul(
                        pv1[:],
                        mm(wv_tiles[c][:]),
                        mm(fm_tiles[c][:, 512:1024]),
                        start=(c == 0),
                        stop=(c == NCHUNK - 1),
                    )
                vt = vpool.tile([128, NPIX + 1], F32, tag="v")
                nc.scalar.activation(
                    out=vt[:, 0:512], in_=pv0[:], func=AF.Identity,
                    bias=bv_t[:, o : o + 1],
                )
                nc.scalar.activation(
                    out=vt[:, 512:1024], in_=pv1[:], func=AF.Identity,
                    bias=bv_t[:, o : o + 1],
                )
                nc.vector.memset(vt[:, 1024:1025], 1.0)
                v_tiles.append(vt)

                # s matvec for this o-chunk (always full fp32)
                wq_tiles = []
                for c in range(NCHUNK):
                    qt = wq_pool.tile([128, 128], F32, tag="wq")
                    nc.sync.dma_start(
                        out=qt[:], in_=wqt[o, c * 128 : (c + 1) * 128, :]
                    )
                    wq_tiles.append(qt)
                ps = pss.tile([128, 2], F32, tag="ps")
                for c in range(NCHUNK):
                    nc.tensor.matmul(
                        ps[:],
                        wq_tiles[c][:],
                        u_tiles[c][:],
                        start=(c == 0),
                        stop=(c == NCHUNK - 1),
                    )
                nc.vector.scalar_tensor_tensor(
                    out=s_cols[:, o : o + 1],
                    in0=bq_t[:, o : o + 1],
                    scalar=a_col[:, 0:1],
                    in1=ps[:, 0:1],
                    op0=OP.mult,
                    op1=OP.add,
                )
                nc.vector.scalar_tensor_tensor(
                    out=s_cols[:, NCHUNK + o : NCHUNK + o + 1],
                    in0=bq_t[:, o : o + 1],
                    scalar=float(NPIX),
                    in1=ps[:, 1:2],
                    op0=OP.mult,
                    op1=OP.add,
                )
                # row max via support points
                hx = hx_pool.tile([128, NH], F32, tag="hx")
                nc.vector.tensor_scalar_mul(hx[:], hull_wk[:], s_cols[:, o : o + 1])
                nc.vector.scalar_tensor_tensor(
                    out=hx[:],
                    in0=hull_bk[:],
                    scalar=s_cols[:, NCHUNK + o : NCHUNK + o + 1],
                    in1=hx[:],
                    op0=OP.mult,
                    op1=OP.add,
                )
                nc.vector.tensor_reduce(
                    out=s_cols[:, 2 * NCHUNK + o : 2 * NCHUNK + o + 1],
                    in_=hx[:],
                    axis=AX.X,
                    op=OP.max,
                )
                # park the three columns in DRAM (partition-major = i order)
                for r in range(3):
                    nc.sync.dma_start(
                        out=scratch[r : r + 1, o * 128 : (o + 1) * 128],
                        in_=s_cols[:, r * NCHUNK + o : r * NCHUNK + o + 1],
                    )

        # ---- phase D: exp(scores^T) blocks + probs @ v ------------------
        with ExitStack() as pd:
            rows = pd.enter_context(tc.tile_pool(name="rows", bufs=1))
            e_pool = pd.enter_context(tc.tile_pool(name="e", bufs=24))
            res_pool = pd.enter_context(tc.tile_pool(name="res", bufs=3))
            o_pool = pd.enter_context(tc.tile_pool(name="osb", bufs=3))
            z_pool = pd.enter_context(tc.tile_pool(name="z", bufs=4))
            pso = pd.enter_context(tc.tile_pool(name="pso", bufs=6, space="PSUM"))

            s1r = rows.tile([128, C], F32, tag="s1r")
            s2r = rows.tile([128, C], F32, tag="s2r")
            m_r = rows.tile([128, C], F32, tag="m_r")
            nc.gpsimd.dma_start(out=s1r[:], in_=scratch[0:1, :].to_broadcast([128, C]))
            nc.gpsimd.dma_start(out=s2r[:], in_=scratch[1:2, :].to_broadcast([128, C]))
            nc.gpsimd.dma_start(out=m_r[:], in_=scratch[2:3, :].to_broadcast([128, C]))

            for ib in range(4):
                isl = slice(ib * 512, (ib + 1) * 512)
                eb = []
                for j in range(NCHUNK):
                    et = e_pool.tile([128, 512], F32, tag="e")
                    # (s2_i * bk_j) - m_i
                    nc.vector.scalar_tensor_tensor(
                        out=et[:],
                        in0=s2r[:, isl],
                        scalar=bk_t[:, j : j + 1],
                        in1=m_r[:, isl],
                        op0=OP.mult,
                        op1=OP.subtract,
                    )
                    # (s1_i * wk_j) + prev
                    nc.vector.scalar_tensor_tensor(
                        out=et[:],
                        in0=s1r[:, isl],
                        scalar=wk_t[:, j : j + 1],
                        in1=et[:],
                        op0=OP.mult,
                        op1=OP.add,
                    )
                    nc.scalar.activation(out=et[:], in_=et[:], func=AF.Exp)
                    eb.append(et)
                for ic in range(4):
                    ig = ib * 4 + ic
                    po = [
                        pso.tile([128, b - a], F32, tag="po", name=f"po{ig}_{a}")
                        for (a, b) in NSPLIT
                    ]
                    for nidx, (a, b) in enumerate(NSPLIT):
                        for j in range(NCHUNK):
                            nc.tensor.matmul(
                                po[nidx][:],
                                mm(eb[j][:, ic * 128 : (ic + 1) * 128]),
                                mm(v_tiles[j][:, a:b]),
                                start=(j == 0),
                                stop=(j == NCHUNK - 1),
                            )
                    rz = z_pool.tile([128, 1], F32, tag="rz")
                    nc.vector.reciprocal(rz[:], po[2][:, 340:341])
                    rzg = z_pool.tile([128, 1], F32, tag="rzg")
                    nc.vector.tensor_mul(rzg[:], rz[:], gam_bc[:])
                    fr = res_pool.tile([128, NPIX], F32, tag="res")
                    nc.sync.dma_start(
                        out=fr[:], in_=fm[ig * 128 : (ig + 1) * 128, :]
                    )
                    ot = o_pool.tile([128, NPIX], F32, tag="osb")
                    spans = [(0, 342, 0), (342, 684, 1), (684, 1024, 2)]
                    for a, b, nidx in spans:
                        nc.vector.scalar_tensor_tensor(
                            out=ot[:, a:b],
                            in0=po[nidx][:, 0 : b - a],
                            scalar=rzg[:, 0:1],
                            in1=fr[:, a:b],
                            op0=OP.mult,
                            op1=OP.add,
                        )
                    nc.sync.dma_start(
                        out=out[ig * 128 : (ig + 1) * 128, :], in_=ot[:]
                    )

    nc.compile()
    return nc


def host_inputs(feature_map, attention_map, Wq, bq, Wk, bk, Wv, bv, gamma):
    """Shard + lay out inputs for the 8 cores; returns in_maps list."""
    f32 = np.float32
    B = feature_map.shape[0]
    fm = np.ascontiguousarray(feature_map.reshape(B, C, NPIX).astype(f32, copy=False))
    am = np.ascontiguousarray(
        attention_map.reshape(B, 1, NPIX).astype(f32, copy=False)
    )
    wqt_blk = np.ascontiguousarray(
        Wq.astype(f32, copy=False).T.reshape(C, NCHUNK, 128).transpose(1, 0, 2)
    )
    wvt_blk = np.ascontiguousarray(
        Wv.astype(f32, copy=False).T.reshape(C, NCHUNK, 128).transpose(1, 0, 2)
    )
    wk1 = Wk.reshape(C).astype(f32, copy=False)
    bk1 = bk.reshape(C).astype(f32, copy=False)
    wk_col = np.ascontiguousarray(wk1.reshape(C, 1))
    bk_col = np.ascontiguousarray(bk1.reshape(C, 1))
    bq_col = np.ascontiguousarray(bq.reshape(C, 1).astype(f32, copy=False))
    bv_col = np.ascontiguousarray(bv.reshape(C, 1).astype(f32, copy=False))

    # direction-sampled support points of {(Wk_j, bk_j)}: subset whose max
    # of (Wk_j * x + bk_j * y) is within r*(1-cos(pi/NH)) of the true max
    th = np.arange(NH, dtype=np.float64) * (2.0 * np.pi / NH)
    proj = np.cos(th)[:, None] * wk1[None, :] + np.sin(th)[:, None] * bk1[None, :]
    sel = np.argmax(proj, axis=1)
    hull = np.ascontiguousarray(np.stack([wk1[sel], bk1[sel]]).astype(f32))

    gam2 = np.ascontiguousarray(gamma.reshape(1, 1).astype(f32, copy=False))

    shared = dict(
        wvt=wvt_blk,
        wqt=wqt_blk,
        wk=wk_col,
        bk=bk_col,
        bq=bq_col,
        bv=bv_col,
        hull=hull,
        gamma=gam2,
    )
    return [dict(fm=fm[b], am=am[b], **shared) for b in range(B)]


_NC_CACHE = {}


def get_nc(mm_dt=MM_DT):
    key = str(mm_dt)
    if key not in _NC_CACHE:
        _NC_CACHE[key] = build_nc(mm_dt)
    return _NC_CACHE[key]


def kernel(feature_map, attention_map, Wq, bq, Wk, bk, Wv, bv, gamma, **run_kwargs):
    from concourse.bass_utils import run_bass_kernel_spmd

    B, _, H, W = feature_map.shape
    in_maps = host_inputs(
        feature_map, attention_map, Wq, bq, Wk, bk, Wv, bv, gamma
    )
    nc = get_nc()
    res = run_bass_kernel_spmd(nc, in_maps, core_ids=list(range(NCORES)), **run_kwargs)
    out = np.stack([res.results[b]["out"].reshape(C, H, W) for b in range(B)])
    if run_kwargs:
        kernel.last_results = res
    return out.astype(np.float32, copy=False)
